# revision 26
# baseline (speedup 1.0000x reference)
"""Multi-head self-attention (S=2048, B=2, D=1024, H=16) on 8 TRN2 NeuronCores.

Sharding: core c handles batch b = c//4 and head-quad g = c%4 (4 heads of 64).
Megatron-style: in_proj column-sliced, out_proj row-sliced; host sums the 8
partial outputs (bf16 partials) and adds the output bias. The V-projection
bias is folded into the host-side output bias (softmax rows sum to 1, so
P@(V + 1*bv) = P@V + 1*bv).

Per-core dataflow (matmul inputs bf16, accumulation fp32):
  - host supplies x^T (D-major) activations and pre-transposed weight slices
  - qpT/kpT computed head-major (m on partitions, seq on free)
  - vp computed seq-major with an interleaved ones column per head (65-wide
    blocks) so the PV matmul also produces softmax row-sums on partition 64
  - scores^T per (head-pair, 512-query-chunk, key-tile) in a packed psum tile
    (128, 2, 512); exp on ACT reads the pair in one op
  - normalization: DVE reciprocal of the row-sums then an SBUF->SBUF DMA
    partition-broadcast; DVE multiplies into attnT

Scheduling: the ACT exp stream (128 x 1147ns = 147us) and the PE stream
(~145us) are the rooflines. All input DMA goes on the single sync HWDGE
queue in strict priority order (the ~350GB/s per-core HBM pipe is shared,
so parallel queues only steal from the critical path). Minimal eager
projection starts attention at ~18us; all other projections and the
out-projection are paced background generators between attention
iterations. Tail copies route to the then-idle ACT engine.
"""

import math
from collections import deque
from contextlib import ExitStack

import numpy as np

S = 2048
B = 2
D = 1024
H = 16
DK = 64
HC = 4          # heads per core
M = HC * DK     # 256 head-dim columns per core
N_CORES = 8
KT = S // 128   # 16 key tiles
QQ = 4          # 512-wide query chunks

MM_DT = "bfloat16"   # dtype of matmul inputs

_compiled = None


def _build_program():
    import concourse.tile as tile
    from concourse import mybir, bacc

    f32 = mybir.dt.float32
    f32r = mybir.dt.float32r
    mdt = getattr(mybir.dt, MM_DT)
    EXP = mybir.ActivationFunctionType.Exp

    nc = bacc.Bacc("TRN2", target_bir_lowering=False, debug=False)

    xqT = nc.dram_tensor("xqT", [D, S], mdt, kind="ExternalInput").ap()
    xkT = nc.dram_tensor("xkT", [D, S], mdt, kind="ExternalInput").ap()
    xvT = nc.dram_tensor("xvT", [D, S], mdt, kind="ExternalInput").ap()
    wqT = nc.dram_tensor("wqT", [D, M], mdt, kind="ExternalInput").ap()
    wkT = nc.dram_tensor("wkT", [D, M], mdt, kind="ExternalInput").ap()
    wvT = nc.dram_tensor("wvT", [D, M], mdt, kind="ExternalInput").ap()
    bq = nc.dram_tensor("bq", [M], f32, kind="ExternalInput").ap()
    bk = nc.dram_tensor("bk", [M], f32, kind="ExternalInput").ap()
    woT = nc.dram_tensor("woT", [M, D], mdt, kind="ExternalInput").ap()
    out = nc.dram_tensor("out", [S, D], mdt, kind="ExternalOutput").ap()

    from concourse import library_config

    with tile.TileContext(nc) as tc, ExitStack() as ctx:
        const_pool = ctx.enter_context(tc.tile_pool(name="const", bufs=1))
        x_pool = ctx.enter_context(tc.tile_pool(name="x", bufs=16))
        e_pool = ctx.enter_context(tc.tile_pool(name="e", bufs=8))
        o_pool = ctx.enter_context(tc.tile_pool(name="o", bufs=3))
        r_pool = ctx.enter_context(tc.tile_pool(name="r", bufs=2))
        ps_a = ctx.enter_context(tc.tile_pool(name="ps_a", bufs=1, space="PSUM"))
        ps_b = ctx.enter_context(tc.tile_pool(name="ps_b", bufs=4, space="PSUM"))

        # ---- persistent SBUF tensors ----
        wq_sb = const_pool.tile([128, 8, M], mdt)
        wk_sb = const_pool.tile([128, 8, M], mdt)
        wv_sb = const_pool.tile([128, 8, M], mdt)
        wo_sb = const_pool.tile([128, 2, D], mdt)
        bq_sb = const_pool.tile([128, 2], f32)
        bk_sb = const_pool.tile([128, 2], f32)

        qpT = const_pool.tile([128, 2, S], mdt)   # [p, mt, s]
        kpT = const_pool.tile([128, 2, S], mdt)
        vp = const_pool.tile([128, KT, HC * 65], mdt)  # aug: 65-wide per head
        attnT = const_pool.tile([128, 2, S], mdt)

        # ones columns of the augmented V (head h at column h*65+64)
        nc.vector.memset(
            vp[:, :, :].rearrange("p kt (h c) -> p kt h c", c=65)[:, :, :, 64:65], 1.0
        )
        # identity for the PE transposes in the softmax epilogue
        from concourse.masks import make_identity

        ident = const_pool.tile([128, 128], mdt)
        make_identity(nc, ident)

        def make_chunk(tag, i):
            return x_pool.tile([128, 1024], mdt, tag=tag, name=f"{tag}{i}")

        def load_cols(x_dr, xt, idx, ns, ne, eng):
            half, kc = divmod(idx, 8)
            fs = half * 1024
            eng.dma_start(
                out=xt[:, ns:ne],
                in_=x_dr[kc * 128:(kc + 1) * 128, fs + ns:fs + ne],
            )
            return xt

        xk_ch = [make_chunk("xk", i) for i in range(16)]
        xq_ch = [make_chunk("xq", i) for i in range(16)]
        xv_ch = [make_chunk("xv", i) for i in range(16)]

        # ---- input DMA in consumption order. The ~350GB/s per-core HBM pipe
        # is shared, so ordering IS prioritization, and whole [128,1024]
        # chunks keep 2KB DMA lines (finer slices measurably throttle the
        # aggregate stream — the entire first phase is DMA-bound). xq-h0
        # rides the scalar HWDGE queue in parallel so its issues don't
        # serialize behind xk's on sync.
        nc.sync.dma_start(
            out=wk_sb[:, :, :], in_=wkT.rearrange("(kc p) m -> p kc m", p=128)
        )
        nc.sync.dma_start(
            out=wq_sb[:, :, :], in_=wqT.rearrange("(kc p) m -> p kc m", p=128)
        )
        for i in range(8):
            load_cols(xkT, xk_ch[i], i, 0, 1024, nc.sync)
        for i in range(8):
            load_cols(xqT, xq_ch[i], i, 0, 1024, nc.scalar)
        # bias loads expand to many tiny descriptors (slow issue) — keep them
        # behind xq on the scalar queue
        nc.scalar.dma_start(out=bk_sb[:, :], in_=bk.rearrange("(mt p) -> p mt", p=128))
        nc.scalar.dma_start(out=bq_sb[:, :], in_=bq.rearrange("(mt p) -> p mt", p=128))
        nc.scalar.dma_start(
            out=wv_sb[:, :, :], in_=wvT.rearrange("(kc p) m -> p kc m", p=128)
        )
        for i in range(8):
            load_cols(xvT, xv_ch[i], i, 0, 1024, nc.sync)
        for i in range(8, 16):
            load_cols(xkT, xk_ch[i], i, 0, 1024, nc.sync)
        for i in range(8, 16):
            load_cols(xvT, xv_ch[i], i, 0, 1024, nc.sync)
        for i in range(8, 16):
            load_cols(xqT, xq_ch[i], i, 0, 1024, nc.sync)
        nc.sync.dma_start(
            out=wo_sb[:, :, :], in_=woT.rearrange("(kc p) j -> p kc j", p=128)
        )

        # ---- projection chains (generators yield ~ns of PE work emitted) ----
        def gen_proj(w_sb, b_sb, p_sb, chunks, mt, half, nch, ns=None, ne=None):
            fs = half * 1024
            if ns is None:
                ns, ne = nch * 512, nch * 512 + 512
            width = ne - ns
            ps = ps_b.tile([128, 512], f32, tag="ps_small",
                           name=f"ps_p{mt}{half}{ns}")
            for kc in range(8):
                nc.tensor.matmul(
                    ps[:, 0:width],
                    w_sb[:, kc, mt * 128:(mt + 1) * 128],
                    chunks[half * 8 + kc][:, ns:ne],
                    start=(kc == 0),
                    stop=(kc == 7),
                )
                if kc < 7:
                    yield 220 * width // 512 + 90
            nc.vector.tensor_scalar_add(
                out=p_sb[:, mt, fs + ns:fs + ne],
                in0=ps[:, 0:width],
                scalar1=b_sb[:, mt:mt + 1],
            )
            yield 500

        # vp super-group: two key-tiles into one PSUM bank (the second chain's
        # first matmul uses start=False: its elements' has_written bits are
        # clear, so it overwrites rather than accumulates — one bank-clear for
        # the whole tile).
        def vp_group2(g):
            ps = ps_b.tile([128, 2, 256], f32, tag="ps_small", name="ps_v")
            for sub in range(2):
                kt = 2 * g + sub
                half, st = divmod(kt, 8)
                for kc in range(8):
                    nc.tensor.matmul(
                        ps[:, sub, 0:M],
                        xv_ch[half * 8 + kc][:, st * 128:(st + 1) * 128],
                        wv_sb[:, kc, :],
                        start=(sub == 0 and kc == 0),
                        stop=(sub == 1 and kc == 7),
                        skip_group_check=True,
                    )
                yield 900
            nc.vector.tensor_copy(
                out=vp[:, 2 * g:2 * g + 2, :]
                    .rearrange("p k (h c) -> p k h c", c=65)[:, :, :, 0:64],
                in_=ps[:, :, 0:M].rearrange("p k (h c) -> p k h c", c=64),
            )
            yield 100

        # ---- softmax epilogue ----
        # PV accumulates query-major (u[hh] = [128 queries, qb, 64 dims + z]),
        # so the row-sum z sits per-query-PARTITION: normalization is a DVE
        # reciprocal + per-partition tensor_scalar multiply (no cross-
        # partition broadcast at all). A cheap PE transpose (identity
        # matmul, 128 rows @ 1cyc/row) restores the head-major attnT layout
        # the out-projection contracts over.
        def flush_q_a(u_new, w=512):
            # phase A, at segment end (DVE only, frees the u PSUM banks):
            # reciprocal of the per-query z column, per-partition multiply
            nqb = w // 128
            rz = r_pool.tile([128, 2, 4], f32, tag="rz")
            atq = r_pool.tile([128, 2, 4, 64], mdt, tag="atq")
            for hh in range(2):
                nc.vector.reciprocal_approx_fast(
                    out=rz[:, hh, 0:nqb], in_=u_new[hh][:, 0:nqb, 64]
                )
            for hh in range(2):
                for qb in range(nqb):
                    with nc.allow_low_precision(reason="softmax normalize"):
                        nc.vector.tensor_scalar_mul(
                            atq[:, hh, qb, :],
                            u_new[hh][:, qb, 0:64],
                            rz[:, hh, qb:qb + 1],
                        )
            return atq

        def flush_q_b(pair, qs, w, atq, use_act=False):
            # phase B, deferred into the next segment: PE transposes back to
            # head-major attnT, then PSUM->SBUF copies
            nqb = w // 128
            tp = ps_b.tile(
                [64, 2, 4, 128], mdt, tag="ps_small", name=f"tp{pair}{qs}"
            )
            for hh in range(2):
                for qb in range(nqb):
                    nc.tensor.matmul(
                        tp[0:64, hh, qb, :],
                        atq[:, hh, qb, :],
                        ident[:, :],
                        is_transpose=True,
                        start=(hh == 0 and qb == 0),
                        stop=(hh == 1 and qb == nqb - 1),
                        skip_group_check=True,
                    )
            for hh in range(2):
                if use_act:
                    nc.scalar.copy(
                        out=attnT[hh * 64:(hh + 1) * 64, pair, qs:qs + w],
                        in_=tp[0:64, hh, 0:nqb, :],
                    )
                else:
                    nc.vector.tensor_copy(
                        out=attnT[hh * 64:(hh + 1) * 64, pair, qs:qs + w],
                        in_=tp[0:64, hh, 0:nqb, :],
                    )

        # ---- out-projection (generator, per 128-row seq tile) ----
        def gen_outproj(sg, use_act=False):
            ot = o_pool.tile([128, D], mdt, tag="ot")
            for nch2 in range(2):
                po = ps_b.tile([128, 512], f32, tag="ps_small", name=f"po{sg}{nch2}")
                for kc in range(2):
                    nc.tensor.matmul(
                        po[:, :],
                        attnT[:, kc, sg * 128:(sg + 1) * 128],
                        wo_sb[:, kc, nch2 * 512:(nch2 + 1) * 512],
                        start=(kc == 0),
                        stop=(kc == 1),
                    )
                if use_act and nch2 == 0:
                    nc.scalar.copy(out=ot[:, nch2 * 512:(nch2 + 1) * 512], in_=po[:, :])
                else:
                    nc.vector.tensor_copy(
                        out=ot[:, nch2 * 512:(nch2 + 1) * 512], in_=po[:, :]
                    )
                yield 700
            # alternate output DMA across the two HWDGE queues so the tail
            # stores drain in parallel
            (nc.sync if sg % 2 == 0 else nc.scalar).dma_start(
                out=out[sg * 128:(sg + 1) * 128, :], in_=ot[:, :]
            )
            yield 100

        # ---- eager startup: just enough projection to start attention ----
        # kp keys 0-511 and qp queries 0-511, chains interleaved so each
        # follows its own DMA stream (xk on sync, xq on scalar) in parallel
        eager = deque([
            gen_proj(wk_sb, bk_sb, kpT, xk_ch, 0, 0, 0),
            gen_proj(wq_sb, bq_sb, qpT, xq_ch, 0, 0, 0),
        ])
        while eager:
            g = eager.popleft()
            try:
                next(g)
                eager.append(g)
            except StopIteration:
                pass

        # background work, ordered by deadline (pair0 carries everything that
        # pair1's first segment needs; the rest rides in pair1's slack)
        bg = deque([
            gen_proj(wk_sb, bk_sb, kpT, xk_ch, 0, 0, 1),  # keys 512-1023 (kt4)
            gen_proj(wk_sb, bk_sb, kpT, xk_ch, 0, 1, 0),  # keys 1024-1535 (qq0 kt8)
            gen_proj(wk_sb, bk_sb, kpT, xk_ch, 0, 1, 1),  # keys 1536-2047 (qq0 kt12)
            gen_proj(wq_sb, bq_sb, qpT, xq_ch, 0, 0, 1),  # q 512-1023 (qq1)
            gen_proj(wq_sb, bq_sb, qpT, xq_ch, 0, 1, 0),  # q 1024-1535 (qq2)
            gen_proj(wq_sb, bq_sb, qpT, xq_ch, 0, 1, 1),  # q 1536-2047 (qq3)
            gen_proj(wk_sb, bk_sb, kpT, xk_ch, 1, 0, 0),  # pair1 keys
            gen_proj(wk_sb, bk_sb, kpT, xk_ch, 1, 0, 1),
            gen_proj(wk_sb, bk_sb, kpT, xk_ch, 1, 1, 0),
            gen_proj(wk_sb, bk_sb, kpT, xk_ch, 1, 1, 1),
            gen_proj(wq_sb, bq_sb, qpT, xq_ch, 1, 0, 0),  # pair1-qq0 queries
        ])
        bg_p1 = [
            gen_proj(wq_sb, bq_sb, qpT, xq_ch, 1, 0, 1),  # pair1-qq1
            gen_proj(wq_sb, bq_sb, qpT, xq_ch, 1, 1, 0),  # pair1-qq2
            gen_proj(wq_sb, bq_sb, qpT, xq_ch, 1, 1, 1),  # pair1-qq3
        ]

        def pump(budget):
            while budget > 0 and bg:
                try:
                    budget -= next(bg[0])
                except StopIteration:
                    bg.popleft()

        # paired scores/exp: two key-tiles share one 4-bank PSUM tile so the
        # exp runs as ONE ACT instruction over 2048 elems/partition, halving
        # the per-instruction ACT overhead (the ACT stream is the pacer).
        # ps_a is a single tile (ring depth 1): scores of the next pair wait
        # on the previous pair's exp, which the ACT stream keeps just ahead.
        sc_cur = [None]

        def scores_exp(pair, qs, kt, w=512):
            if kt % 2 == 0:
                sc_cur[0] = ps_a.tile(
                    [128, 4, 512], f32, tag="ps_main", name="sc_pair"
                )
            sc = sc_cur[0]
            base = 2 * (kt % 2)
            ks = kt * 128
            for hh in range(2):
                po = hh * 64
                nc.tensor.matmul(
                    sc[:, base + hh, 0:w],
                    kpT[po:po + 64, pair, ks:ks + 128],
                    qpT[po:po + 64, pair, qs:qs + w],
                    start=True,
                    stop=True,
                )
            if kt % 2 == 0:
                return None
            et = e_pool.tile([128, 4, 512], mdt, tag="et")
            nc.scalar.activation(
                out=et[:, :, 0:w], in_=sc[:, :, 0:w], func=EXP
            )
            return et

        # query-major PV: out[128 queries, 65] per (head, 128-query block),
        # contraction over the full 128 keys of the tile — 100% PE array
        # utilization and only 65 streamed rows per matmul. The vp ones
        # column lands the softmax row-sum z in output column 64, on the
        # QUERY partition where the normalize needs it.
        def pv(pair, u_tiles, kt, et, w=512):
            nqb = w // 128
            sub = 2 * (kt % 2)
            for hh in range(2):
                h = 2 * pair + hh
                for qb in range(nqb):
                    nc.tensor.matmul(
                        u_tiles[hh][:, qb, :],
                        et[:, sub + hh, qb * 128:(qb + 1) * 128],
                        vp[:, kt, h * 65:(h + 1) * 65],
                        start=(kt == 0 and qb == 0),
                        stop=(kt == KT - 1 and qb == nqb - 1),
                        skip_group_check=True,
                    )

        def new_u(tag):
            return [
                ps_b.tile([128, 4, 65], f32, tag="ps_small", name=f"u_{tag}{hh}")
                for hh in range(2)
            ]

        # ---- first segment (pair0, qq0): vp is built here just-in-time,
        # one key-tile sub-chain per iteration. PV lags the exp stream by
        # PV_LAG iterations so DMA-gated vp work never sits ahead of ready
        # work in the in-order PE queue; the PV drain extends past kt15,
        # pulling vp groups in lockstep so late xv DMA never stalls scores.
        PV_LAG = 7
        vq = deque(vp_group2(g) for g in range(8))
        vdone = [0]  # count of fully emitted vp groups

        def vpump(budget):
            while budget > 0 and vq:
                try:
                    budget -= next(vq[0])
                except StopIteration:
                    vq.popleft()
                    vdone[0] += 1

        def vpump_until(gmax):
            while vq and vdone[0] <= gmax:
                try:
                    next(vq[0])
                except StopIteration:
                    vq.popleft()
                    vdone[0] += 1

        # ---- unified segment pipeline ----
        # One PV per iteration, lagged LAGQ iterations behind the scores/exp
        # stream; a segment's last LAGQ PVs ride the next segment's
        # iterations, so scores/exp never pause at segment boundaries (ACT
        # stays the pacer) and the first segment's DMA-gated vp build drains
        # under qq1's compute. flush_q_b (PE transposes + attnT copies) and
        # the dependent out-projection generators are emitted only after the
        # owning segment's phase-A normalize, keeping the in-order engine
        # queues cycle-free. The last qq3/pair1 segment is split into two
        # 256-query halves so the final normalize + out-projection tail is
        # half as long.
        LAGQ = 8
        all_segs = [(0, 0, 512), (0, 512, 512), (0, 1024, 512),
                    (0, 1536, 512), (1, 0, 512), (1, 512, 512),
                    (1, 1024, 512), (1, 1536, 256), (1, 1792, 256)]
        prev = None
        for si, (pair, qs, w) in enumerate(all_segs):
            st = {"pair": pair, "qs": qs, "w": w, "u": None, "ets": {},
                  "first": si == 0, "atq": None,
                  "outproj": ([12, 13] if (pair, qs) == (1, 1536) else
                              list(range(qs // 128, qs // 128 + 4))
                              if pair == 1 and w == 512 else [])}
            def seg_pv(state, k0):
                if state["first"]:
                    vpump_until(k0 // 2)
                etp = state["ets"][k0 // 2]
                pv(state["pair"], state["u"], k0, etp, state["w"])
                if k0 % 2 == 1:
                    state["ets"].pop(k0 // 2)

            for kt in range(KT):
                etp = scores_exp(pair, qs, kt, w)
                if etp is not None:
                    st["ets"][kt // 2] = etp
                if si == 0:
                    pump(900 if kt <= 8 else 700)
                    if kt >= 5:
                        vpump(950)
                else:
                    if kt == LAGQ + 1 and prev is not None:
                        flush_q_b(prev["pair"], prev["qs"], prev["w"],
                                  prev["atq"])
                        for sg in prev["outproj"]:
                            bg.append(gen_outproj(sg))
                        prev = None
                    if kt >= 2:
                        pump(650 if pair == 1 else 450)
                # one PV per iteration: previous segment's leftovers first
                if prev is not None and kt < LAGQ:
                    seg_pv(prev, KT - LAGQ + kt)
                    if kt == LAGQ - 1:
                        prev["atq"] = flush_q_a(prev["u"], prev["w"])
                elif kt >= LAGQ:
                    if st["u"] is None:
                        st["u"] = new_u(f"{pair}q{qs}")
                    st["pv_fn"] = seg_pv
                    seg_pv(st, kt - LAGQ)
            if pair == 0 and qs == 1536:
                bg.extend(bg_p1)
            prev = st
        # tail: drain the last segment's lagged PVs, then flush and the
        # final two seq tiles (leftover bg hides the flush latency)
        for kt in range(LAGQ):
            prev["pv_fn"](prev, KT - LAGQ + kt)
            pump(600)
        atq = flush_q_a(prev["u"], prev["w"])
        while bg:
            pump(1 << 30)
        flush_q_b(prev["pair"], prev["qs"], prev["w"], atq, use_act=True)
        tail_gens = deque([gen_outproj(14, use_act=True),
                           gen_outproj(15, use_act=True)])
        while tail_gens:
            g = tail_gens.popleft()
            try:
                next(g)
                tail_gens.append(g)
            except StopIteration:
                pass

    nc.compile()
    return nc


def _get_compiled():
    global _compiled
    if _compiled is None:
        _compiled = _build_program()
    return _compiled


def _make_in_maps(q, k, v, in_proj_w, in_proj_b, out_proj_w):
    import ml_dtypes

    mdt_np = np.dtype(ml_dtypes.bfloat16) if MM_DT == "bfloat16" else np.float32

    def cvt(a):
        return np.ascontiguousarray(a).astype(mdt_np)

    xT = {}
    for b in range(B):
        xT[b] = (
            cvt(q[:, b, :].T),
            cvt(k[:, b, :].T),
            cvt(v[:, b, :].T),
        )
    scale = 1.0 / math.sqrt(DK)
    in_maps = []
    for c in range(N_CORES):
        b, g = divmod(c, HC)
        cols = slice(g * M, (g + 1) * M)
        in_maps.append({
            "xqT": xT[b][0],
            "xkT": xT[b][1],
            "xvT": xT[b][2],
            "wqT": cvt((in_proj_w[0 * D:1 * D][cols] * scale).T),
            "wkT": cvt(in_proj_w[1 * D:2 * D][cols].T),
            "wvT": cvt(in_proj_w[2 * D:3 * D][cols].T),
            "bq": np.ascontiguousarray(in_proj_b[0 * D:1 * D][cols] * scale),
            "bk": np.ascontiguousarray(in_proj_b[1 * D:2 * D][cols]),
            "woT": cvt(out_proj_w[:, g * M:(g + 1) * M].T),
        })
    return in_maps


def kernel(q, k, v, in_proj_w, in_proj_b, out_proj_w, out_proj_b):
    from concourse.bass_utils import run_bass_kernel_spmd

    q = np.asarray(q, dtype=np.float32)
    k = np.asarray(k, dtype=np.float32)
    v = np.asarray(v, dtype=np.float32)
    in_proj_w = np.asarray(in_proj_w, dtype=np.float32)
    in_proj_b = np.asarray(in_proj_b, dtype=np.float32)
    out_proj_w = np.asarray(out_proj_w, dtype=np.float32)
    out_proj_b = np.asarray(out_proj_b, dtype=np.float32)

    nc = _get_compiled()
    in_maps = _make_in_maps(q, k, v, in_proj_w, in_proj_b, out_proj_w)

    res = run_bass_kernel_spmd(nc, in_maps, core_ids=list(range(N_CORES)))

    # V-projection bias folded here: softmax rows sum to 1, so the bv term
    # contributes out_proj_w @ bv to every output row.
    bias = out_proj_b + out_proj_w @ in_proj_b[2 * D:3 * D]
    out = np.broadcast_to(bias.astype(np.float32), (S, B, D)).copy()
    for c in range(N_CORES):
        out[:, c // HC, :] += res.results[c]["out"].astype(np.float32)
    return out



# revision 31
# speedup vs baseline: 1.2967x; 1.2967x over previous
"""Multi-head self-attention (S=2048, B=2, D=1024, H=16) on 8 TRN2 NeuronCores.

Sharding: core c handles batch b = c//4 and head-quad g = c%4 (4 heads of 64).
Megatron-style: in_proj column-sliced, out_proj row-sliced; host sums the 8
partial outputs (bf16 partials) and adds the output bias. The V-projection
bias is folded into the host-side output bias (softmax rows sum to 1, so
P@(V + 1*bv) = P@V + 1*bv).

Per-core dataflow (matmul inputs bf16, accumulation fp32):
  - host supplies x^T (D-major) activations and pre-transposed weight slices
  - qpT/kpT computed head-major (m on partitions, seq on free)
  - vp computed seq-major with an interleaved ones column per head (65-wide
    blocks) so the PV matmul also produces softmax row-sums on partition 64
  - scores^T per (head-pair, 512-query-chunk, key-tile) in a packed psum tile
    (128, 2, 512); exp on ACT reads the pair in one op
  - normalization: DVE reciprocal of the row-sums then an SBUF->SBUF DMA
    partition-broadcast; DVE multiplies into attnT

Scheduling: the ACT exp stream (128 x 1147ns = 147us) and the PE stream
(~145us) are the rooflines. All input DMA goes on the single sync HWDGE
queue in strict priority order (the ~350GB/s per-core HBM pipe is shared,
so parallel queues only steal from the critical path). Minimal eager
projection starts attention at ~18us; all other projections and the
out-projection are paced background generators between attention
iterations. Tail copies route to the then-idle ACT engine.
"""

import math
from collections import deque
from contextlib import ExitStack

import numpy as np

S = 2048
B = 2
D = 1024
H = 16
DK = 64
HC = 4          # heads per core
M = HC * DK     # 256 head-dim columns per core
N_CORES = 8
KT = S // 128   # 16 key tiles
QQ = 4          # 512-wide query chunks

MM_DT = "bfloat16"   # dtype of matmul inputs

_compiled = None


def _build_program():
    import concourse.tile as tile
    from concourse import mybir, bacc

    f32 = mybir.dt.float32
    f32r = mybir.dt.float32r
    mdt = getattr(mybir.dt, MM_DT)
    EXP = mybir.ActivationFunctionType.Exp

    nc = bacc.Bacc("TRN2", target_bir_lowering=False, debug=False)

    xqT = nc.dram_tensor("xqT", [D, S], mdt, kind="ExternalInput").ap()
    xkT = nc.dram_tensor("xkT", [D, S], mdt, kind="ExternalInput").ap()
    xvT = nc.dram_tensor("xvT", [D, S], mdt, kind="ExternalInput").ap()
    wqT = nc.dram_tensor("wqT", [D, M], mdt, kind="ExternalInput").ap()
    wkT = nc.dram_tensor("wkT", [D, M], mdt, kind="ExternalInput").ap()
    wvT = nc.dram_tensor("wvT", [D, M], mdt, kind="ExternalInput").ap()
    bq = nc.dram_tensor("bq", [M], f32, kind="ExternalInput").ap()
    bk = nc.dram_tensor("bk", [M], f32, kind="ExternalInput").ap()
    woT = nc.dram_tensor("woT", [M, D], mdt, kind="ExternalInput").ap()
    out = nc.dram_tensor("out", [S, D], mdt, kind="ExternalOutput").ap()

    from concourse import library_config

    with tile.TileContext(nc) as tc, ExitStack() as ctx:
        const_pool = ctx.enter_context(tc.tile_pool(name="const", bufs=1))
        x_pool = ctx.enter_context(tc.tile_pool(name="x", bufs=16))
        e_pool = ctx.enter_context(tc.tile_pool(name="e", bufs=12))
        o_pool = ctx.enter_context(tc.tile_pool(name="o", bufs=3))
        r_pool = ctx.enter_context(tc.tile_pool(name="r", bufs=2))
        ps_a = ctx.enter_context(tc.tile_pool(name="ps_a", bufs=2, space="PSUM"))
        ps_b = ctx.enter_context(tc.tile_pool(name="ps_b", bufs=4, space="PSUM"))

        # ---- persistent SBUF tensors ----
        wq_sb = const_pool.tile([128, 8, M], mdt)
        wk_sb = const_pool.tile([128, 8, M], mdt)
        wv_sb = const_pool.tile([128, 8, M], mdt)
        wo_sb = const_pool.tile([128, 2, D], mdt)
        bq_sb = const_pool.tile([128, 2], f32)
        bk_sb = const_pool.tile([128, 2], f32)

        qpT = const_pool.tile([128, 2, S], mdt)   # [p, mt, s]
        kpT = const_pool.tile([128, 2, S], mdt)
        vp = const_pool.tile([128, KT, HC * 65], mdt)  # aug: 65-wide per head
        attnT = const_pool.tile([128, 2, S], mdt)

        # ones columns of the augmented V (head h at column h*65+64)
        nc.vector.memset(
            vp[:, :, :].rearrange("p kt (h c) -> p kt h c", c=65)[:, :, :, 64:65], 1.0
        )
        # identity for the PE transposes in the softmax epilogue
        from concourse.masks import make_identity

        ident = const_pool.tile([128, 128], mdt)
        make_identity(nc, ident)

        def make_chunk(tag, i):
            return x_pool.tile([128, 1024], mdt, tag=tag, name=f"{tag}{i}")

        def load_cols(x_dr, xt, idx, ns, ne, eng):
            half, kc = divmod(idx, 8)
            fs = half * 1024
            eng.dma_start(
                out=xt[:, ns:ne],
                in_=x_dr[kc * 128:(kc + 1) * 128, fs + ns:fs + ne],
            )
            return xt

        xk_ch = [make_chunk("xk", i) for i in range(16)]
        xq_ch = [make_chunk("xq", i) for i in range(16)]
        xv_ch = [make_chunk("xv", i) for i in range(16)]

        # ---- input DMA in consumption order. The ~350GB/s per-core HBM pipe
        # is shared, so ordering IS prioritization, and whole [128,1024]
        # chunks keep 2KB DMA lines (finer slices measurably throttle the
        # aggregate stream — the entire first phase is DMA-bound). xq-h0
        # rides the scalar HWDGE queue in parallel so its issues don't
        # serialize behind xk's on sync.
        nc.sync.dma_start(
            out=wk_sb[:, :, :], in_=wkT.rearrange("(kc p) m -> p kc m", p=128)
        )
        nc.sync.dma_start(
            out=wq_sb[:, :, :], in_=wqT.rearrange("(kc p) m -> p kc m", p=128)
        )
        for i in range(8):
            load_cols(xkT, xk_ch[i], i, 0, 1024, nc.sync)
        for i in range(8):
            load_cols(xqT, xq_ch[i], i, 0, 1024, nc.scalar)
        # bias loads expand to many tiny descriptors (slow issue) — keep them
        # behind xq on the scalar queue
        nc.scalar.dma_start(out=bk_sb[:, :], in_=bk.rearrange("(mt p) -> p mt", p=128))
        nc.scalar.dma_start(out=bq_sb[:, :], in_=bq.rearrange("(mt p) -> p mt", p=128))
        nc.scalar.dma_start(
            out=wv_sb[:, :, :], in_=wvT.rearrange("(kc p) m -> p kc m", p=128)
        )
        for i in range(8, 16):
            load_cols(xkT, xk_ch[i], i, 0, 1024, nc.sync)
        for i in range(8):
            load_cols(xvT, xv_ch[i], i, 0, 1024, nc.sync)
        for i in range(8, 16):
            load_cols(xvT, xv_ch[i], i, 0, 1024, nc.sync)
        for i in range(8, 16):
            load_cols(xqT, xq_ch[i], i, 0, 1024, nc.sync)
        nc.sync.dma_start(
            out=wo_sb[:, :, :], in_=woT.rearrange("(kc p) j -> p kc j", p=128)
        )

        # ---- projection chains (generators yield ~ns of PE work emitted) ----
        def gen_proj(w_sb, b_sb, p_sb, chunks, mt, half, nch, ns=None, ne=None):
            fs = half * 1024
            if ns is None:
                ns, ne = nch * 512, nch * 512 + 512
            width = ne - ns
            ps = ps_b.tile([128, 512], f32, tag="ps_small",
                           name=f"ps_p{mt}{half}{ns}")
            for kc in range(8):
                nc.tensor.matmul(
                    ps[:, 0:width],
                    w_sb[:, kc, mt * 128:(mt + 1) * 128],
                    chunks[half * 8 + kc][:, ns:ne],
                    start=(kc == 0),
                    stop=(kc == 7),
                )
                if kc < 7:
                    yield 220 * width // 512 + 90
            nc.vector.tensor_scalar_add(
                out=p_sb[:, mt, fs + ns:fs + ne],
                in0=ps[:, 0:width],
                scalar1=b_sb[:, mt:mt + 1],
            )
            yield 500

        # vp super-group: two key-tiles into one PSUM bank (the second chain's
        # first matmul uses start=False: its elements' has_written bits are
        # clear, so it overwrites rather than accumulates — one bank-clear for
        # the whole tile).
        def vp_group2(g):
            ps = ps_b.tile([128, 2, 256], f32, tag="ps_small", name="ps_v")
            for sub in range(2):
                kt = 2 * g + sub
                half, st = divmod(kt, 8)
                for kc in range(8):
                    nc.tensor.matmul(
                        ps[:, sub, 0:M],
                        xv_ch[half * 8 + kc][:, st * 128:(st + 1) * 128],
                        wv_sb[:, kc, :],
                        start=(sub == 0 and kc == 0),
                        stop=(sub == 1 and kc == 7),
                        skip_group_check=True,
                    )
                yield 900
            nc.vector.tensor_copy(
                out=vp[:, 2 * g:2 * g + 2, :]
                    .rearrange("p k (h c) -> p k h c", c=65)[:, :, :, 0:64],
                in_=ps[:, :, 0:M].rearrange("p k (h c) -> p k h c", c=64),
            )
            yield 100

        # ---- softmax epilogue ----
        # PV accumulates query-major (u[hh] = [128 queries, qb, 64 dims + z]),
        # so the row-sum z sits per-query-PARTITION: normalization is a DVE
        # reciprocal + per-partition tensor_scalar multiply (no cross-
        # partition broadcast at all). A cheap PE transpose (identity
        # matmul, 128 rows @ 1cyc/row) restores the head-major attnT layout
        # the out-projection contracts over.
        def flush_q_a(u_new, w=512):
            # phase A, at segment end (DVE only, frees the u PSUM banks):
            # reciprocal of the per-query z column, per-partition multiply
            nqb = w // 128
            rz = r_pool.tile([128, 2, 4], f32, tag="rz")
            atq = r_pool.tile([128, 2, 4, 64], mdt, tag="atq")
            for hh in range(2):
                nc.vector.reciprocal_approx_fast(
                    out=rz[:, hh, 0:nqb], in_=u_new[hh][:, 0:nqb, 64]
                )
            for hh in range(2):
                for qb in range(nqb):
                    with nc.allow_low_precision(reason="softmax normalize"):
                        nc.vector.tensor_scalar_mul(
                            atq[:, hh, qb, :],
                            u_new[hh][:, qb, 0:64],
                            rz[:, hh, qb:qb + 1],
                        )
            return atq

        def flush_q_b(pair, qs, w, atq, use_act=False):
            # phase B, deferred into the next segment: PE transposes back to
            # head-major attnT, then PSUM->SBUF copies
            nqb = w // 128
            tp = ps_b.tile(
                [64, 2, 4, 128], mdt, tag="ps_small", name=f"tp{pair}{qs}"
            )
            for hh in range(2):
                for qb in range(nqb):
                    nc.tensor.matmul(
                        tp[0:64, hh, qb, :],
                        atq[:, hh, qb, :],
                        ident[:, :],
                        is_transpose=True,
                        start=(hh == 0 and qb == 0),
                        stop=(hh == 1 and qb == nqb - 1),
                        skip_group_check=True,
                    )
            for hh in range(2):
                if use_act:
                    nc.scalar.copy(
                        out=attnT[hh * 64:(hh + 1) * 64, pair, qs:qs + w],
                        in_=tp[0:64, hh, 0:nqb, :],
                    )
                else:
                    nc.vector.tensor_copy(
                        out=attnT[hh * 64:(hh + 1) * 64, pair, qs:qs + w],
                        in_=tp[0:64, hh, 0:nqb, :],
                    )

        # ---- out-projection (generator, per 128-row seq tile) ----
        def gen_outproj(sg, use_act=False):
            ot = o_pool.tile([128, D], mdt, tag="ot")
            for nch2 in range(2):
                po = ps_b.tile([128, 512], f32, tag="ps_small", name=f"po{sg}{nch2}")
                for kc in range(2):
                    nc.tensor.matmul(
                        po[:, :],
                        attnT[:, kc, sg * 128:(sg + 1) * 128],
                        wo_sb[:, kc, nch2 * 512:(nch2 + 1) * 512],
                        start=(kc == 0),
                        stop=(kc == 1),
                    )
                if use_act and nch2 == 0:
                    nc.scalar.copy(out=ot[:, nch2 * 512:(nch2 + 1) * 512], in_=po[:, :])
                else:
                    nc.vector.tensor_copy(
                        out=ot[:, nch2 * 512:(nch2 + 1) * 512], in_=po[:, :]
                    )
                yield 700
            # alternate output DMA across the two HWDGE queues so the tail
            # stores drain in parallel
            (nc.sync if sg % 2 == 0 else nc.scalar).dma_start(
                out=out[sg * 128:(sg + 1) * 128, :], in_=ot[:, :]
            )
            yield 100

        # ---- eager startup: just enough projection to start attention ----
        # kp keys 0-511 and qp queries 0-511, chains interleaved so each
        # follows its own DMA stream (xk on sync, xq on scalar) in parallel
        eager = deque([
            gen_proj(wk_sb, bk_sb, kpT, xk_ch, 0, 0, 0),
            gen_proj(wq_sb, bq_sb, qpT, xq_ch, 0, 0, 0),
        ])
        while eager:
            g = eager.popleft()
            try:
                next(g)
                eager.append(g)
            except StopIteration:
                pass

        # background work, ordered by deadline (pair0 carries everything that
        # pair1's first segment needs; the rest rides in pair1's slack)
        bg = deque([
            gen_proj(wk_sb, bk_sb, kpT, xk_ch, 0, 0, 1),  # keys 512-1023 (kt4)
            gen_proj(wk_sb, bk_sb, kpT, xk_ch, 0, 1, 0),  # keys 1024-1535 (qq0 kt8)
            gen_proj(wk_sb, bk_sb, kpT, xk_ch, 0, 1, 1),  # keys 1536-2047 (qq0 kt12)
            gen_proj(wq_sb, bq_sb, qpT, xq_ch, 0, 0, 1),  # q 512-1023 (qq1)
            gen_proj(wq_sb, bq_sb, qpT, xq_ch, 0, 1, 0),  # q 1024-1535 (qq2)
            gen_proj(wq_sb, bq_sb, qpT, xq_ch, 0, 1, 1),  # q 1536-2047 (qq3)
            gen_proj(wk_sb, bk_sb, kpT, xk_ch, 1, 0, 0),  # pair1 keys
            gen_proj(wk_sb, bk_sb, kpT, xk_ch, 1, 0, 1),
            gen_proj(wk_sb, bk_sb, kpT, xk_ch, 1, 1, 0),
            gen_proj(wk_sb, bk_sb, kpT, xk_ch, 1, 1, 1),
            gen_proj(wq_sb, bq_sb, qpT, xq_ch, 1, 0, 0),  # pair1-qq0 queries
        ])
        bg_p1 = [
            gen_proj(wq_sb, bq_sb, qpT, xq_ch, 1, 0, 1),  # pair1-qq1
            gen_proj(wq_sb, bq_sb, qpT, xq_ch, 1, 1, 0),  # pair1-qq2
            gen_proj(wq_sb, bq_sb, qpT, xq_ch, 1, 1, 1),  # pair1-qq3
        ]

        def pump(budget):
            while budget > 0 and bg:
                try:
                    budget -= next(bg[0])
                except StopIteration:
                    bg.popleft()

        def scores_exp(pair, qs, kt, w=512):
            # tiles are always allocated full-width (same pool tag => same
            # PSUM bank budget); narrow segments just use a column prefix
            sc = ps_a.tile([128, 2, 512], f32, tag="ps_main")
            ks = kt * 128
            for hh in range(2):
                po = hh * 64
                nc.tensor.matmul(
                    sc[:, hh, 0:w],
                    kpT[po:po + 64, pair, ks:ks + 128],
                    qpT[po:po + 64, pair, qs:qs + w],
                    start=True,
                    stop=True,
                )
            et = e_pool.tile([128, 2, 512], mdt, tag="et")
            nc.scalar.activation(
                out=et[:, :, 0:w], in_=sc[:, :, 0:w], func=EXP
            )
            return et

        # query-major PV: out[128 queries, 65] per (head, 128-query block),
        # contraction over the full 128 keys of the tile — 100% PE array
        # utilization and only 65 streamed rows per matmul. The vp ones
        # column lands the softmax row-sum z in output column 64, on the
        # QUERY partition where the normalize needs it.
        def pv(pair, u_tiles, kt, et, w=512):
            nqb = w // 128
            for hh in range(2):
                h = 2 * pair + hh
                for qb in range(nqb):
                    nc.tensor.matmul(
                        u_tiles[hh][:, qb, :],
                        et[:, hh, qb * 128:(qb + 1) * 128],
                        vp[:, kt, h * 65:(h + 1) * 65],
                        start=(kt == 0 and qb == 0),
                        stop=(kt == KT - 1 and qb == nqb - 1),
                        skip_group_check=True,
                    )

        def new_u(tag):
            return [
                ps_b.tile([128, 4, 65], f32, tag="ps_small", name=f"u_{tag}{hh}")
                for hh in range(2)
            ]

        # ---- first segment (pair0, qq0): vp is built here just-in-time,
        # one key-tile sub-chain per iteration. PV lags the exp stream by
        # PV_LAG iterations so DMA-gated vp work never sits ahead of ready
        # work in the in-order PE queue; the PV drain extends past kt15,
        # pulling vp groups in lockstep so late xv DMA never stalls scores.
        PV_LAG = 7
        vq = deque(vp_group2(g) for g in range(8))
        vdone = [0]  # count of fully emitted vp groups

        def vpump(budget):
            while budget > 0 and vq:
                try:
                    budget -= next(vq[0])
                except StopIteration:
                    vq.popleft()
                    vdone[0] += 1

        def vpump_until(gmax):
            while vq and vdone[0] <= gmax:
                try:
                    next(vq[0])
                except StopIteration:
                    vq.popleft()
                    vdone[0] += 1

        # ---- unified segment pipeline ----
        # One PV per iteration, lagged LAGQ iterations behind the scores/exp
        # stream; a segment's last LAGQ PVs ride the next segment's
        # iterations, so scores/exp never pause at segment boundaries (ACT
        # stays the pacer) and the first segment's DMA-gated vp build drains
        # under qq1's compute. flush_q_b (PE transposes + attnT copies) and
        # the dependent out-projection generators are emitted only after the
        # owning segment's phase-A normalize, keeping the in-order engine
        # queues cycle-free. The last qq3/pair1 segment is split into two
        # 256-query halves so the final normalize + out-projection tail is
        # half as long.
        LAGQ = 8
        all_segs = [(0, 0, 512), (0, 512, 512), (0, 1024, 512),
                    (0, 1536, 512), (1, 0, 512), (1, 512, 512),
                    (1, 1024, 512), (1, 1536, 256), (1, 1792, 256)]
        prev = None
        for si, (pair, qs, w) in enumerate(all_segs):
            st = {"pair": pair, "qs": qs, "w": w, "u": None, "ets": {},
                  "first": si == 0, "atq": None,
                  "outproj": ([12, 13] if (pair, qs) == (1, 1536) else
                              list(range(qs // 128, qs // 128 + 4))
                              if pair == 1 and w == 512 else [])}
            def seg_pv(state, k0):
                if state["first"]:
                    vpump_until(k0 // 2)
                pv(state["pair"], state["u"], k0, state["ets"].pop(k0),
                   state["w"])

            for kt in range(KT):
                st["ets"][kt] = scores_exp(pair, qs, kt, w)
                if si == 0:
                    pump(900 if kt <= 8 else 700)
                    if kt >= 5:
                        vpump(950)
                else:
                    if kt == LAGQ + 1 and prev is not None:
                        flush_q_b(prev["pair"], prev["qs"], prev["w"],
                                  prev["atq"])
                        for sg in prev["outproj"]:
                            bg.append(gen_outproj(sg))
                        prev = None
                    if kt >= 2:
                        pump(650 if pair == 1 else 450)
                # one PV per iteration: previous segment's leftovers first
                if prev is not None and kt < LAGQ:
                    seg_pv(prev, KT - LAGQ + kt)
                    if kt == LAGQ - 1:
                        prev["atq"] = flush_q_a(prev["u"], prev["w"])
                elif kt >= LAGQ:
                    if st["u"] is None:
                        st["u"] = new_u(f"{pair}q{qs}")
                    st["pv_fn"] = seg_pv
                    seg_pv(st, kt - LAGQ)
            if pair == 0 and qs == 1536:
                bg.extend(bg_p1)
            prev = st
        # tail: drain the last segment's lagged PVs, then flush and the
        # final two seq tiles (leftover bg hides the flush latency)
        for kt in range(LAGQ):
            prev["pv_fn"](prev, KT - LAGQ + kt)
            pump(600)
        atq = flush_q_a(prev["u"], prev["w"])
        while bg:
            pump(1 << 30)
        flush_q_b(prev["pair"], prev["qs"], prev["w"], atq, use_act=True)
        tail_gens = deque([gen_outproj(14, use_act=True),
                           gen_outproj(15, use_act=True)])
        while tail_gens:
            g = tail_gens.popleft()
            try:
                next(g)
                tail_gens.append(g)
            except StopIteration:
                pass

    nc.compile()
    return nc


def _get_compiled():
    global _compiled
    if _compiled is None:
        _compiled = _build_program()
    return _compiled


def _make_in_maps(q, k, v, in_proj_w, in_proj_b, out_proj_w):
    import ml_dtypes

    mdt_np = np.dtype(ml_dtypes.bfloat16) if MM_DT == "bfloat16" else np.float32

    def cvt(a):
        return np.ascontiguousarray(a).astype(mdt_np)

    xT = {}
    for b in range(B):
        xT[b] = (
            cvt(q[:, b, :].T),
            cvt(k[:, b, :].T),
            cvt(v[:, b, :].T),
        )
    scale = 1.0 / math.sqrt(DK)
    in_maps = []
    for c in range(N_CORES):
        b, g = divmod(c, HC)
        cols = slice(g * M, (g + 1) * M)
        in_maps.append({
            "xqT": xT[b][0],
            "xkT": xT[b][1],
            "xvT": xT[b][2],
            "wqT": cvt((in_proj_w[0 * D:1 * D][cols] * scale).T),
            "wkT": cvt(in_proj_w[1 * D:2 * D][cols].T),
            "wvT": cvt(in_proj_w[2 * D:3 * D][cols].T),
            "bq": np.ascontiguousarray(in_proj_b[0 * D:1 * D][cols] * scale),
            "bk": np.ascontiguousarray(in_proj_b[1 * D:2 * D][cols]),
            "woT": cvt(out_proj_w[:, g * M:(g + 1) * M].T),
        })
    return in_maps


def kernel(q, k, v, in_proj_w, in_proj_b, out_proj_w, out_proj_b):
    from concourse.bass_utils import run_bass_kernel_spmd

    q = np.asarray(q, dtype=np.float32)
    k = np.asarray(k, dtype=np.float32)
    v = np.asarray(v, dtype=np.float32)
    in_proj_w = np.asarray(in_proj_w, dtype=np.float32)
    in_proj_b = np.asarray(in_proj_b, dtype=np.float32)
    out_proj_w = np.asarray(out_proj_w, dtype=np.float32)
    out_proj_b = np.asarray(out_proj_b, dtype=np.float32)

    nc = _get_compiled()
    in_maps = _make_in_maps(q, k, v, in_proj_w, in_proj_b, out_proj_w)

    res = run_bass_kernel_spmd(nc, in_maps, core_ids=list(range(N_CORES)))

    # V-projection bias folded here: softmax rows sum to 1, so the bv term
    # contributes out_proj_w @ bv to every output row.
    bias = out_proj_b + out_proj_w @ in_proj_b[2 * D:3 * D]
    out = np.broadcast_to(bias.astype(np.float32), (S, B, D)).copy()
    for c in range(N_CORES):
        out[:, c // HC, :] += res.results[c]["out"].astype(np.float32)
    return out

